# revision 17
# baseline (speedup 1.0000x reference)
"""Trainium2 Bass kernel for nn_AttentionFold (self-contained).

Data-parallel over batch N=16: core i processes clouds {2i, 2i+1}.
Feature-major layout on-chip: activations stored [feature, point].

Key algebraic restructurings vs the reference:
  - glob (512-dim) contribution to gate/fold hidden layers is a per-cloud
    constant -> computed once per cloud as a matvec, applied as relu bias.
  - softmax bias sb2 folded into the filters: E = exp(z), F' = F * exp(sb2),
    sumexp = exp(sb2) . E  (softmax is shift/scale invariant in this form).
  - sigmoid(x) = 0.5 + 0.5*tanh(x/2) so the whole kernel uses one ACT
    table set (exp_and_others: exp/tanh/relu/identity/square).
  - coords grid is input-independent -> host constant, pre-transposed.
  - normalization scale 1/sqrt(max||c||^2) via DVE Newton rsqrt (no sqrt
    table set switch).
"""

import numpy as np

import concourse.bass as bass
import concourse.tile as tile
from concourse import bacc, mybir
from concourse import bass_utils

F32 = mybir.dt.float32
F32R = mybir.dt.float32r
U32 = mybir.dt.uint32
AF = mybir.ActivationFunctionType
ALU = mybir.AluOpType

N, P, C, HW, G = 16, 4096, 128, 784, 512
NCORES = 8
CPC = N // NCORES          # clouds per core = 2
NCH = P // 512             # chunks per cloud = 8
CH = 512                   # points per chunk
QT = [128, 128, 128, 128, 128, 128, 16]   # q tiles of HW=784
K_GRID = 64


def _build_program():
    """Build + compile the per-core Bass program once. Returns nc."""
    nc = bacc.Bacc("TRN2", target_bir_lowering=False, debug=False,
                   num_devices=NCORES)

    dt_in = {}

    def din(name, shape, dt=F32):
        dt_in[name] = nc.dram_tensor(name, shape, dt, kind="ExternalInput").ap()
        return dt_in[name]

    pts_d = din("pts", (CPC, P, 3), F32R)
    xf_d = din("xf", (CPC, P, 12), F32R)
    filt_d = din("filt", (CPC, C, HW))
    glob_d = din("glob", (CPC, G))
    w1aug_d = din("w1aug", (17, 384), F32R)
    sw2_d = din("sw2", (128, HW), F32R)
    expb_d = din("expb", (128, 7), F32R)
    expbf_d = din("expbf", (128, 7))
    gw1g_d = din("gw1g", (128, 512))
    fw1g_d = din("fw1g", (128, 512))
    fw1s_d = din("fw1s", (128, 128), F32R)
    gw2_d = din("gw2", (128, 128), F32R)
    gb2h_d = din("gb2h", (128, 1))
    fw2_d = din("fw2", (128, 128), F32R)
    fb2_d = din("fb2", (128, 1))
    fw3_d = din("fw3", (128, 3), F32R)
    fb3_d = din("fb3", (3, 1))
    sb1_d = din("sb1", (128, 1))
    gb1_d = din("gb1", (128, 1))
    fb1_d = din("fb1", (128, 1))
    coordsT_d = din("coordsT", (2, P), F32R)
    ident_d = din("ident", (128, 128))
    ident3_d = din("ident3", (3, 3), F32R)
    bc05_d = din("bc05", (1, 128), F32R)
    ones13_d = din("ones13", (1, 3))
    ones3_d = din("ones3", (3, 1))

    out_d = nc.dram_tensor("out", (CPC, P, 3), F32, kind="ExternalOutput").ap()

    with tile.TileContext(nc) as tc:
        from contextlib import ExitStack
        with ExitStack() as ctx:
            cpool = ctx.enter_context(tc.tile_pool(name="consts", bufs=1))
            clpool = ctx.enter_context(tc.tile_pool(name="cloud", bufs=2))
            cl1pool = ctx.enter_context(tc.tile_pool(name="cloud1", bufs=1))
            xpool = ctx.enter_context(tc.tile_pool(name="x", bufs=3))
            spool = ctx.enter_context(tc.tile_pool(name="acts", bufs=2))
            epool = ctx.enter_context(tc.tile_pool(name="e", bufs=2))
            # PSUM budget: 8 banks = z(2 banks x 2 bufs) + w(1 bank x 3) + sm(1)
            pz = ctx.enter_context(tc.tile_pool(name="pz", bufs=2, space="PSUM"))
            pw = ctx.enter_context(tc.tile_pool(name="pw", bufs=3, space="PSUM"))
            psm = ctx.enter_context(tc.tile_pool(name="psm", bufs=1, space="PSUM"))

            def cload(name, dram, shape, dt=F32):
                t = cpool.tile(shape, dt, tag=name)
                nc.sync.dma_start(t[:], dram[:])
                return t

            w1aug = cload("w1aug", w1aug_d, [17, 384], F32R)
            sw2 = cload("sw2", sw2_d, [128, HW], F32R)
            expb = cload("expb", expb_d, [128, 7], F32R)
            expbf = cload("expbf", expbf_d, [128, 7])
            gw1g = cload("gw1g", gw1g_d, [128, 512])
            fw1g = cload("fw1g", fw1g_d, [128, 512])
            fw1s = cload("fw1s", fw1s_d, [128, 128], F32R)
            gw2 = cload("gw2", gw2_d, [128, 128], F32R)
            gb2h = cload("gb2h", gb2h_d, [128, 1])
            fw2 = cload("fw2", fw2_d, [128, 128], F32R)
            fb2 = cload("fb2", fb2_d, [128, 1])
            fw3 = cload("fw3", fw3_d, [128, 3], F32R)
            fb3 = cload("fb3", fb3_d, [3, 1])
            sb1 = cload("sb1", sb1_d, [128, 1])
            gb1 = cload("gb1", gb1_d, [128, 1])
            fb1 = cload("fb1", fb1_d, [128, 1])
            ident = cload("ident", ident_d, [128, 128])
            ident3 = cload("ident3", ident3_d, [3, 3], F32R)
            bc05 = cload("bc05", bc05_d, [1, 128], F32R)
            ones13 = cload("ones13", ones13_d, [1, 3])
            ones3 = cload("ones3", ones3_d, [3, 1])
            rsqC = cpool.tile([1, 1], U32, tag="rsqC")
            nc.vector.memset(rsqC[:], 0x5F3759DF)

            for n in range(CPC):
                # ---- per-cloud prep ----
                Fsb = clpool.tile([128, HW], F32, tag="Fsb")
                nc.sync.dma_start(Fsb[:], filt_d[n])
                glob_sb = clpool.tile([128, 4], F32, tag="glob")
                nc.sync.dma_start(
                    glob_sb[:], glob_d[n].rearrange("(c p) -> p c", p=128))

                # F' = (F * exp(sb2)) transposed -> FT7 [q, c] tiles
                FT7 = clpool.tile([128, 896], F32R, tag="FT7")
                for j in range(7):
                    q = QT[j]
                    ftp = pw.tile([128, 128], F32, tag="w")
                    nc.tensor.transpose(
                        ftp[0:q, :], Fsb[:, 128 * j:128 * j + q], ident[:])
                    nc.vector.tensor_scalar_mul(
                        FT7[0:q, 128 * j:128 * (j + 1)], ftp[0:q, :],
                        expbf[0:q, j:j + 1])

                # glob matvecs -> per-cloud gate/fold biases
                gps = pw.tile([128, 1], F32, tag="w")
                for j in range(4):
                    nc.tensor.matmul(
                        gps[:], gw1g[:, 128 * j:128 * (j + 1)],
                        glob_sb[:, j:j + 1], start=(j == 0), stop=(j == 3))
                gbias = clpool.tile([128, 1], F32, tag="gbias")
                nc.vector.tensor_tensor(gbias[:], gps[:], gb1[:], ALU.add)
                fps = pw.tile([128, 1], F32, tag="w")
                for j in range(4):
                    nc.tensor.matmul(
                        fps[:], fw1g[:, 128 * j:128 * (j + 1)],
                        glob_sb[:, j:j + 1], start=(j == 0), stop=(j == 3))
                fbias = clpool.tile([128, 1], F32, tag="fbias")
                nc.vector.tensor_tensor(fbias[:], fps[:], fb1[:], ALU.add)

                opre = clpool.tile([3, P], F32, tag="opre")
                msum = clpool.tile([3, NCH], F32, tag="msum")

                # ---- per-chunk pipeline ----
                for c in range(NCH):
                    sl = slice(CH * c, CH * (c + 1))
                    X17 = xpool.tile([17, CH], F32R, tag="X17")
                    nc.sync.dma_start(
                        X17[0:3, :], pts_d[n, sl, :].rearrange("p c -> c p"))
                    nc.sync.dma_start(
                        X17[3:15, :], xf_d[n, sl, :].rearrange("p c -> c p"))
                    nc.sync.dma_start(X17[15:17, :], coordsT_d[:, sl])

                    sh_ps = pw.tile([128, CH], F32, tag="w")
                    nc.tensor.matmul(sh_ps[:], w1aug[:, 0:128],
                                     X17[:], start=True, stop=True)
                    gh_ps = pw.tile([128, CH], F32, tag="w")
                    nc.tensor.matmul(gh_ps[:], w1aug[:, 128:256],
                                     X17[:], start=True, stop=True)
                    sh = spool.tile([128, CH], F32R, tag="sh")
                    nc.vector.tensor_scalar(sh[:], sh_ps[:], sb1[:], 0.0,
                                            ALU.add, ALU.max)
                    gh = spool.tile([128, CH], F32R, tag="gh")
                    nc.vector.tensor_scalar(gh[:], gh_ps[:], gbias[:],
                                            0.0, ALU.add, ALU.max)

                    E = epool.tile([128, 3584], F32R, tag="E")
                    for r, js in enumerate([(0, 1), (2, 3), (4, 5), (6,)]):
                        zt = pz.tile([128, 1024], F32, tag="z")
                        for i, j in enumerate(js):
                            q = QT[j]
                            nc.tensor.matmul(
                                zt[0:q, 512 * i:512 * i + 512],
                                sw2[:, 128 * j:128 * j + q], sh[:],
                                start=True, stop=True)
                        w = 512 * len(js)
                        nc.scalar.activation(
                            E[:, 1024 * r:1024 * r + w], zt[:, 0:w], AF.Exp)

                    spat = pw.tile([128, CH], F32, tag="w")
                    sume = psm.tile([1, CH], F32, tag="sm")
                    for j in range(7):
                        q = QT[j]
                        esl = E[0:q, 512 * j:512 * (j + 1)]
                        nc.tensor.matmul(
                            spat[:], FT7[0:q, 128 * j:128 * (j + 1)],
                            esl, start=(j == 0), stop=(j == 6))
                        nc.tensor.matmul(
                            sume[:], expb[0:q, j:j + 1], esl,
                            start=(j == 0), stop=(j == 6))

                    ga = pw.tile([128, CH], F32, tag="w")
                    nc.tensor.matmul(ga[:], gw2[:], gh[:],
                                     start=True, stop=True)
                    gt = spool.tile([128, CH], F32, tag="gt")
                    nc.scalar.activation(gt[:], ga[:], AF.Tanh,
                                         bias=gb2h[:], scale=0.5)

                    rinv = spool.tile([1, CH], F32R, tag="rinv")
                    with nc.allow_low_precision(reason="fp32r rounding only"):
                        nc.vector.reciprocal(rinv[:], sume[:])
                    rbc = pw.tile([128, CH], F32, tag="w")
                    nc.tensor.matmul(rbc[:], bc05[:], rinv[:],
                                     start=True, stop=True)

                    g1 = spool.tile([128, CH], F32, tag="g1")
                    nc.vector.scalar_tensor_tensor(
                        g1[:], gt[:], 1.0, spat[:], ALU.add, ALU.mult)
                    feats = spool.tile([128, CH], F32R, tag="feats")
                    nc.vector.tensor_tensor(feats[:], g1[:], rbc[:], ALU.mult)

                    f1ps = pw.tile([128, CH], F32, tag="w")
                    nc.tensor.matmul(f1ps[:], w1aug[:, 256:384], X17[:],
                                     start=True, stop=False)
                    nc.tensor.matmul(f1ps[:], fw1s[:], feats[:],
                                     start=False, stop=True)
                    f1 = spool.tile([128, CH], F32R, tag="f1s")
                    nc.vector.tensor_scalar(f1[:], f1ps[:], fbias[:], 0.0,
                                            ALU.add, ALU.max)
                    f2ps = pw.tile([128, CH], F32, tag="w")
                    nc.tensor.matmul(f2ps[:], fw2[:], f1[:],
                                     start=True, stop=True)
                    f2 = spool.tile([128, CH], F32R, tag="f2s")
                    nc.vector.tensor_scalar(f2[:], f2ps[:], fb2[:], 0.0,
                                            ALU.add, ALU.max)
                    f3ps = psm.tile([3, CH], F32, tag="sm")
                    nc.tensor.matmul(f3ps[:], fw3[:], f2[:],
                                     start=True, stop=False)
                    nc.tensor.matmul(f3ps[:], ident3[:], X17[0:3, :],
                                     start=False, stop=True)
                    nc.scalar.activation(opre[:, sl], f3ps[:], AF.Identity,
                                         bias=fb3[:],
                                         accum_out=msum[:, c:c + 1])

                # ---- per-cloud normalize ----
                msr = spool.tile([3, 1], F32, tag="msr")
                nc.vector.reduce_sum(msr[:], msum[:], axis=mybir.AxisListType.X)
                negmean = spool.tile([3, 1], F32, tag="negmean")
                nc.vector.tensor_scalar_mul(negmean[:], msr[:], -1.0 / P)
                sqc = cl1pool.tile([3, P], F32, tag="sqc")
                nc.scalar.activation(sqc[:], opre[:], AF.Square,
                                     bias=negmean[:], scale=1.0)
                n2 = psm.tile([128, 32], F32, tag="sm")
                for c in range(NCH):
                    for b in range(4):
                        nc.tensor.matmul(
                            n2[:, 4 * c + b:4 * c + b + 1],
                            sqc[:, 512 * c + 128 * b:512 * c + 128 * (b + 1)],
                            ones3[:], start=True, stop=True)
                nm128 = spool.tile([128, 1], F32, tag="nm128")
                nc.vector.reduce_max(nm128[:], n2[:], axis=mybir.AxisListType.X)
                nmT = psm.tile([1, 128], F32, tag="sm")
                nc.tensor.transpose(nmT[:], nm128[:], ident[:])
                nmax = spool.tile([1, 1], F32, tag="nmax")
                nc.vector.reduce_max(nmax[:], nmT[:], axis=mybir.AxisListType.X)

                # Newton rsqrt: y ~= 1/sqrt(nmax)
                ysh = spool.tile([1, 1], U32, tag="ysh")
                nc.vector.tensor_scalar(ysh[:], nmax[:].bitcast(U32), 1, None,
                                        ALU.logical_shift_right)
                y = spool.tile([1, 1], F32, tag="y")
                nc.vector.tensor_tensor(y[:].bitcast(U32), rsqC[:], ysh[:],
                                        ALU.subtract)
                t = spool.tile([1, 1], F32, tag="t")
                for _ in range(4):
                    nc.vector.tensor_tensor(t[:], y[:], y[:], ALU.mult)
                    nc.vector.tensor_tensor(t[:], t[:], nmax[:], ALU.mult)
                    nc.vector.tensor_scalar(t[:], t[:], -0.5, 1.5,
                                            ALU.mult, ALU.add)
                    nc.vector.tensor_tensor(y[:], y[:], t[:], ALU.mult)

                rcolps = psm.tile([3, 1], F32, tag="sm")
                nc.tensor.matmul(rcolps[:], ones13[:], y[:],
                                 start=True, stop=True)
                rcol = spool.tile([3, 1], F32, tag="rcol")
                nc.vector.tensor_copy(rcol[:], rcolps[:])
                negmr = spool.tile([3, 1], F32, tag="negmr")
                nc.vector.tensor_tensor(negmr[:], negmean[:], rcol[:], ALU.mult)
                fin = cl1pool.tile([3, P], F32, tag="fin")
                nc.scalar.activation(fin[:], opre[:], AF.Identity,
                                     bias=negmr[:], scale=rcol[:])
                nc.sync.dma_start(out_d[n].rearrange("p c -> c p"), fin[:])

    nc.compile()
    return nc


_prog = None


def _get_prog():
    global _prog
    if _prog is None:
        _prog = _build_program()
    return _prog


def _spatial_grid(k):
    xs = np.linspace(-1.0, 1.0, k, dtype=np.float32)
    gx, gy = np.meshgrid(xs, xs, indexing="ij")
    return np.stack([gx.ravel(), gy.ravel()], axis=-1)


def _host_prep(inputs):
    """Per-call host constants derived from the (input) weights."""
    f32 = np.float32
    sw1 = np.asarray(inputs["sw1"], f32)
    sb1 = np.asarray(inputs["sb1"], f32)
    sw2 = np.asarray(inputs["sw2"], f32)
    sb2 = np.asarray(inputs["sb2"], f32)
    gw1 = np.asarray(inputs["gw1"], f32)
    gb1 = np.asarray(inputs["gb1"], f32)
    gw2 = np.asarray(inputs["gw2"], f32)
    gb2 = np.asarray(inputs["gb2"], f32)
    fw1 = np.asarray(inputs["fw1"], f32)
    fb1 = np.asarray(inputs["fb1"], f32)
    fw2 = np.asarray(inputs["fw2"], f32)
    fb2 = np.asarray(inputs["fb2"], f32)
    fw3 = np.asarray(inputs["fw3"], f32)
    fb3 = np.asarray(inputs["fb3"], f32)

    w1aug = np.zeros((17, 384), f32)
    w1aug[0:15, 0:128] = sw1
    w1aug[0:15, 128:256] = gw1[0:15]
    w1aug[0:15, 256:384] = fw1[0:15]
    w1aug[15:17, 256:384] = fw1[527:529]

    expb = np.zeros((128, 7), f32)
    eb = np.exp(sb2).astype(f32)
    for j in range(7):
        q = QT[j]
        expb[0:q, j] = eb[128 * j:128 * j + q]

    gw1g = np.concatenate(
        [gw1[15 + 128 * j:15 + 128 * (j + 1)] for j in range(4)], axis=1)
    fw1g = np.concatenate(
        [fw1[15 + 128 * j:15 + 128 * (j + 1)] for j in range(4)], axis=1)

    coordsT = np.ascontiguousarray(_spatial_grid(K_GRID).T)

    consts = {
        "w1aug": w1aug,
        "sw2": np.ascontiguousarray(sw2),
        "expb": expb,
        "expbf": expb,
        "gw1g": np.ascontiguousarray(gw1g),
        "fw1g": np.ascontiguousarray(fw1g),
        "fw1s": np.ascontiguousarray(fw1[529:657]),
        "gw2": np.ascontiguousarray(gw2),
        "gb2h": (0.5 * gb2).reshape(128, 1),
        "fw2": np.ascontiguousarray(fw2),
        "fb2": fb2.reshape(128, 1),
        "fw3": np.ascontiguousarray(fw3),
        "fb3": fb3.reshape(3, 1),
        "sb1": sb1.reshape(128, 1),
        "gb1": gb1.reshape(128, 1),
        "fb1": fb1.reshape(128, 1),
        "coordsT": coordsT,
        "ident": np.eye(128, dtype=f32),
        "ident3": np.eye(3, dtype=f32),
        "bc05": np.full((1, 128), 0.5, f32),
        "ones13": np.ones((1, 3), f32),
        "ones3": np.ones((3, 1), f32),
    }
    return {k: np.ascontiguousarray(v, f32) for k, v in consts.items()}


def _in_maps(inputs):
    f32 = np.float32
    pts = np.asarray(inputs["points"], f32)
    xf = np.asarray(inputs["transform"], f32)
    filt = np.asarray(inputs["enc_filters"], f32).reshape(N, C, HW)
    glob = np.asarray(inputs["enc_glob"], f32)
    consts = _host_prep(inputs)
    maps = []
    for i in range(NCORES):
        s = slice(CPC * i, CPC * (i + 1))
        m = {
            "pts": np.ascontiguousarray(pts[s]),
            "xf": np.ascontiguousarray(xf[s]),
            "filt": np.ascontiguousarray(filt[s]),
            "glob": np.ascontiguousarray(glob[s]),
        }
        m.update(consts)
        maps.append(m)
    return maps


def run(inputs, trace=False):
    nc = _get_prog()
    maps = _in_maps(inputs)
    res = bass_utils.run_bass_kernel_spmd(
        nc, maps, core_ids=list(range(NCORES)), trace=trace)
    out = np.concatenate([res.results[i]["out"] for i in range(NCORES)],
                         axis=0)
    return out.astype(np.float32), res


def kernel(**inputs):
    out, _ = run(inputs, trace=False)
    return out
